# revision 1
# baseline (speedup 1.0000x reference)
"""Trainium2 Bass kernel for nn_ANN_Net_146028888292 (dense_mlp, 8 cores).

Strategy (pure data parallel over the batch):
  - Each core gets 524288 rows of x (4194304 / 8).
  - Rows are processed in 16 "big tiles" of 32768 rows (1 MiB f32), DMA'd as
    [128 partitions x 2048 f32] with an f32->bf16 cast during the SWDGE DMA.
  - Compute is feature-major: each [128,128] sub-chunk is PE-transposed so
    partitions hold (group, feature) and batch lives in the free dim.
    Weights are host-side block-diagonal replicas (16 groups of 8 features for
    the 8-wide layers, 32 groups of 4 for the middle layers), bf16.
  - All biases are folded (host-side, exact linear algebra) into the bias
    operand of the next Prelu activation, so residual adds are pure adds.
  - Outputs are produced batch-major directly via activation-as-stationary
    matmuls (lhsT = activations), so stores are contiguous 1 MiB DMAs.
"""

import sys

sys.path.insert(0, "/opt/trn_rl_repo")

import os
from contextlib import ExitStack

import numpy as np
import ml_dtypes

import concourse.bass as bass
import concourse.tile as tile
from concourse import bacc, mybir
from concourse.bass_utils import run_bass_kernel_spmd

F32 = mybir.dt.float32
BF16 = mybir.dt.bfloat16
PRELU = mybir.ActivationFunctionType.Prelu
ALPHA = 0.0025  # rrelu eval slope = (0.001 + 0.004) / 2

N_CORES = 8
B_TOTAL = 4194304
R = B_TOTAL // N_CORES  # 524288 rows per core
BIG_ROWS = 32768  # rows per big tile (1 MiB of f32 x-data)
NT = R // BIG_ROWS  # 16 big tiles per core
N_ROUNDS = 2  # rounds per big tile
SUB_PER_ROUND = 8  # [128,128] sub-chunks per round

_MID_NAMES = ["h2", "h3", "h4", "h5", "h7", "h8", "h9", "h10"]

_cache = {}


def _f64(a):
    return np.asarray(a, dtype=np.float64)


def _bf16(a):
    return np.asarray(a, dtype=np.float32).astype(ml_dtypes.bfloat16)


def _build_consts(inputs):
    """Host-side: block-diagonal weights (bf16) + folded biases (f32)."""
    W = {n: _f64(inputs[f"W_{n}"]) for n in
         ["in", "h1", "h2", "h3", "h4", "h5", "encode", "h6", "h7", "h8", "h9",
          "h10", "decode"]}
    b = {n: _f64(inputs[f"b_{n}"]) for n in W}

    # bias folding through the no-bias residual chains
    c1 = b["h1"]
    beta2 = b["h2"] + c1 @ W["h2"]
    c2 = c1 + b["h3"]
    beta4 = b["h4"] + c2 @ W["h4"]
    c3 = c2 + b["h5"]
    beta_e = float((b["encode"] + c3 @ W["encode"])[0])
    beta9 = b["h9"] + b["h8"] @ W["h9"]
    c6 = b["h8"] + b["h10"]
    beta_d = b["decode"] + c6 @ W["decode"]  # (8,)

    # partition index helpers
    # L8 layout: q = r*8 + f          (16 groups x 8 feats)
    # M  layout: v = 64*h + g*4 + f4  (2 halves x 16 groups x 4 feats)
    # E  layout: e = 16*h + g
    bd_in = np.zeros((128, 128))
    for r in range(16):
        bd_in[r * 8:r * 8 + 8, r * 8:r * 8 + 8] = W["in"]

    bd_h1 = np.zeros((128, 64))
    for r in range(16):
        bd_h1[r * 8:r * 8 + 8, r * 4:r * 4 + 4] = W["h1"]

    def bd_mid(w):
        m = np.zeros((128, 128))
        for v in range(32):
            m[v * 4:v * 4 + 4, v * 4:v * 4 + 4] = w
        return m

    def vec_mid(x4):
        return np.tile(np.asarray(x4, dtype=np.float64), 32)

    w_encb = np.zeros((128, 32))
    for h in range(2):
        for g in range(16):
            w_encb[64 * h + 4 * g:64 * h + 4 * g + 4, 16 * h + g] = W["encode"][:, 0]

    w_h6 = np.zeros((32, 128))
    for h in range(2):
        for g in range(16):
            w_h6[16 * h + g, 64 * h + 4 * g:64 * h + 4 * g + 4] = W["h6"][0, :]

    w_dec = np.zeros((128, 256))
    for h in range(2):
        for g in range(16):
            w_dec[64 * h + 4 * g:64 * h + 4 * g + 4,
                  128 * h + 8 * g:128 * h + 8 * g + 8] = W["decode"]

    consts = {
        "bd_in": _bf16(bd_in),
        "bd_h1": _bf16(bd_h1),
        "w_encb": _bf16(w_encb),
        "w_h6": _bf16(w_h6),
        "w_dec": _bf16(w_dec),
        "ident": _bf16(np.eye(128)),
        "b_in_v": np.tile(_f64(b["in"]), 16).astype(np.float32),
        "beta2_v": vec_mid(beta2).astype(np.float32),
        "beta4_v": vec_mid(beta4).astype(np.float32),
        "betae_v": np.full(128, beta_e, dtype=np.float32),
        "b6_v": vec_mid(b["h6"]).astype(np.float32),
        "b7_v": vec_mid(b["h7"]).astype(np.float32),
        "beta9_v": vec_mid(beta9).astype(np.float32),
        "betad_t": np.tile(beta_d.astype(np.float32), (128, 1)),
        "zero_v": np.zeros(128, dtype=np.float32),
    }
    for n in _MID_NAMES:
        consts[f"bd_{n}"] = _bf16(bd_mid(W[n]))
    return consts


def _build_nc():
    nc = bacc.Bacc("TRN2", target_bir_lowering=False, debug=False,
                   num_devices=N_CORES)

    x_d = nc.dram_tensor("x", [R * 8], F32, kind="ExternalInput").ap()
    enc_d = nc.dram_tensor("enc", [R], F32, kind="ExternalOutput").ap()
    dec_d = nc.dram_tensor("dec", [R * 8], F32, kind="ExternalOutput").ap()

    w_shapes = {
        "bd_in": ([128, 128], BF16), "bd_h1": ([128, 64], BF16),
        "w_encb": ([128, 32], BF16), "w_h6": ([32, 128], BF16),
        "w_dec": ([128, 256], BF16), "ident": ([128, 128], BF16),
        "b_in_v": ([128], F32), "beta2_v": ([128], F32),
        "beta4_v": ([128], F32), "betae_v": ([128], F32),
        "b6_v": ([128], F32), "b7_v": ([128], F32), "beta9_v": ([128], F32),
        "betad_t": ([128, 8], F32), "zero_v": ([128], F32),
    }
    for n in _MID_NAMES:
        w_shapes[f"bd_{n}"] = ([128, 128], BF16)
    w_d = {n: nc.dram_tensor(n, shp, dt, kind="ExternalInput").ap()
           for n, (shp, dt) in w_shapes.items()}

    with tile.TileContext(nc) as tc, ExitStack() as ctx:
        consts = ctx.enter_context(tc.tile_pool(name="consts", bufs=1))
        p_xb = ctx.enter_context(tc.tile_pool(name="xb", bufs=2))
        p_xt = ctx.enter_context(tc.tile_pool(name="xt", bufs=2))
        p_a1 = ctx.enter_context(tc.tile_pool(name="a1", bufs=2))
        p_mch = ctx.enter_context(tc.tile_pool(name="mch", bufs=2))
        p_wd = ctx.enter_context(tc.tile_pool(name="wd", bufs=2))
        p_deco = ctx.enter_context(tc.tile_pool(name="deco", bufs=2))
        p_enco = ctx.enter_context(tc.tile_pool(name="enco", bufs=2))

        ps_t = ctx.enter_context(tc.tile_pool(name="ps_t", bufs=1, space="PSUM"))
        ps_in = ctx.enter_context(tc.tile_pool(name="ps_in", bufs=1, space="PSUM"))
        ps_mid = ctx.enter_context(tc.tile_pool(name="ps_mid", bufs=2, space="PSUM"))
        ps_enc = ctx.enter_context(tc.tile_pool(name="ps_enc", bufs=1, space="PSUM"))
        ps_dec = ctx.enter_context(tc.tile_pool(name="ps_dec", bufs=1, space="PSUM"))

        # --- load constants once ---
        w = {}
        for n, (shp, dt) in w_shapes.items():
            t_ = consts.tile(list(shp) if len(shp) == 2 else [shp[0], 1], dt,
                             tag=n)
            src = w_d[n]
            if len(shp) == 1:
                src = src.rearrange("(p one) -> p one", one=1)
            nc.sync.dma_start(out=t_, in_=src)
            w[n] = t_

        def mm(out, lhsT, rhs):
            nc.tensor.matmul(out, lhsT, rhs, start=True, stop=True)

        def act(out, in_, bias_v):
            nc.scalar.activation(out, in_, PRELU, bias=bias_v, scale=1.0,
                                 alpha=ALPHA)

        for t in range(NT):
            xb = p_xb.tile([128, 2048], BF16, tag="xb")
            nc.gpsimd.dma_start(
                out=xb,
                in_=x_d[t * 262144:(t + 1) * 262144].rearrange(
                    "(p c) -> p c", p=128))
            dec_out = p_deco.tile([128, 2048], F32, tag="dec_out")
            enc_out = p_enco.tile([128, 256], F32, tag="enc_out")

            for r in range(N_ROUNDS):
                # -- transpose 8 sub-chunks into feature-major
                tp = ps_t.tile([128, 1024], BF16, tag="tp")
                for kk in range(SUB_PER_ROUND):
                    k = r * SUB_PER_ROUND + kk
                    nc.tensor.transpose(tp[:, 128 * kk:128 * kk + 128],
                                        xb[:, 128 * k:128 * k + 128],
                                        w["ident"])
                xt = p_xt.tile([128, 1024], BF16, tag="xt")
                nc.vector.tensor_copy(xt, tp)

                # -- in layer (8->8)
                zin = ps_in.tile([128, 1024], F32, tag="zin")
                mm(zin[:, 0:512], w["bd_in"], xt[:, 0:512])
                mm(zin[:, 512:1024], w["bd_in"], xt[:, 512:1024])
                a1 = p_a1.tile([128, 1024], BF16, tag="a1")
                act(a1, zin, w["b_in_v"])

                # -- h1 (8->4) into merged M layout
                x1z = ps_mid.tile([128, 512], F32, tag="z")
                for j in range(4):
                    for h in range(2):
                        kk = 2 * j + h
                        nc.tensor.matmul(
                            x1z[64 * h:64 * h + 64, 128 * j:128 * j + 128],
                            w["bd_h1"], a1[:, 128 * kk:128 * kk + 128],
                            start=True, stop=True)
                x1 = p_mch.tile([128, 512], BF16, tag="x1")
                nc.vector.tensor_copy(x1, x1z)

                # -- pre-encode residual chain
                z2 = ps_mid.tile([128, 512], F32, tag="z")
                mm(z2, w["bd_h2"], x1)
                a2 = p_mch.tile([128, 512], BF16, tag="a2")
                act(a2, z2, w["beta2_v"])

                z3 = ps_mid.tile([128, 512], F32, tag="z")
                mm(z3, w["bd_h3"], a2)
                x2 = p_mch.tile([128, 512], BF16, tag="x2")
                nc.vector.tensor_add(x2, z3, x1)

                z4 = ps_mid.tile([128, 512], F32, tag="z")
                mm(z4, w["bd_h4"], x2)
                a4 = p_mch.tile([128, 512], BF16, tag="a4")
                act(a4, z4, w["beta4_v"])

                z5 = ps_mid.tile([128, 512], F32, tag="z")
                mm(z5, w["bd_h5"], a4)
                x3 = p_mch.tile([128, 512], BF16, tag="x3")
                nc.vector.tensor_add(x3, z5, x2)

                # -- encode (4->1), feature-major for h6
                ze = ps_enc.tile([32, 512], F32, tag="enc")
                mm(ze, w["w_encb"], x3)
                aenc = p_mch.tile([32, 512], BF16, tag="aenc")
                act(aenc, ze, w["betae_v"][0:32])

                # -- encode, batch-major for output (x3 stationary)
                zebm = ps_enc.tile([128, 128], F32, tag="enc")
                for j in range(4):
                    mm(zebm[:, 32 * j:32 * j + 32],
                       x3[:, 128 * j:128 * j + 128], w["w_encb"])
                act(enc_out[:, 128 * r:128 * r + 128], zebm, w["betae_v"])

                # -- h6 (1->4)
                z6 = ps_mid.tile([128, 512], F32, tag="z")
                mm(z6, w["w_h6"], aenc)
                x4 = p_mch.tile([128, 512], BF16, tag="x4")
                act(x4, z6, w["b6_v"])

                # -- post-encode residual chain
                z7 = ps_mid.tile([128, 512], F32, tag="z")
                mm(z7, w["bd_h7"], x4)
                a7 = p_mch.tile([128, 512], BF16, tag="a7")
                act(a7, z7, w["b7_v"])

                z8 = ps_mid.tile([128, 512], F32, tag="z")
                mm(z8, w["bd_h8"], a7)
                x5 = p_mch.tile([128, 512], BF16, tag="x5")
                nc.vector.tensor_add(x5, z8, x4)

                z9 = ps_mid.tile([128, 512], F32, tag="z")
                mm(z9, w["bd_h9"], x5)
                a9 = p_mch.tile([128, 512], BF16, tag="a9")
                act(a9, z9, w["beta9_v"])

                z10 = ps_mid.tile([128, 512], F32, tag="z")
                mm(z10, w["bd_h10"], a9)
                x6 = p_mch.tile([128, 512], BF16, tag="x6")
                nc.vector.tensor_add(x6, z10, x5)

                # -- decode (4->8), batch-major (x6 stationary)
                zd = ps_dec.tile([128, 1024], F32, tag="zd")
                for j in range(4):
                    mm(zd[:, 256 * j:256 * j + 256],
                       x6[:, 128 * j:128 * j + 128], w["w_dec"])
                wd = p_wd.tile([128, 1024], BF16, tag="wd")
                bcast = bass.AP(
                    tensor=w["betad_t"].tensor,
                    offset=w["betad_t"].offset,
                    ap=[w["betad_t"].ap[0], [0, 128], [1, 8]])
                nc.vector.tensor_add(wd, zd, bcast)
                act(dec_out[:, 1024 * r:1024 * r + 1024], wd, w["zero_v"])

            nc.sync.dma_start(
                out=dec_d[t * 262144:(t + 1) * 262144].rearrange(
                    "(p c) -> p c", p=128),
                in_=dec_out)
            nc.sync.dma_start(
                out=enc_d[t * 32768:(t + 1) * 32768].rearrange(
                    "(p c) -> p c", p=128),
                in_=enc_out)

    nc.compile()
    return nc


last_results = None


def kernel(**inputs):
    global last_results
    if "nc" not in _cache:
        _cache["nc"] = _build_nc()
    nc = _cache["nc"]

    consts = _build_consts(inputs)
    x = np.ascontiguousarray(inputs["x_in"], dtype=np.float32)

    in_maps = []
    for c in range(N_CORES):
        m = {"x": np.ascontiguousarray(x[c * R:(c + 1) * R]).reshape(R * 8)}
        m.update(consts)
        in_maps.append(m)

    trace = bool(int(os.environ.get("KERNEL_TRACE", "0")))
    res = run_bass_kernel_spmd(nc, in_maps, core_ids=list(range(N_CORES)),
                               trace=trace)
    last_results = res

    enc = np.concatenate([res.results[c]["enc"] for c in range(N_CORES)])
    dec = np.concatenate([res.results[c]["dec"] for c in range(N_CORES)])
    return (enc.reshape(B_TOTAL, 1), dec.reshape(B_TOTAL, 8))
